# revision 24
# baseline (speedup 1.0000x reference)
"""Distributed 2-layer GAT (BangaloreGAT) on 8 TRN2 NeuronCores — v4.

v2 baseline plus low-risk wins (engine/sync patterns kept identical to v2,
which is stable across hundreds of executions; more aggressive restructures
showed intermittent DMA/engine races on this stack):
- d1 (layer-1 dst attention term per own node) is computed on the host and
  shipped in the blob (50 KB), removing v2's 49-iteration device prologue
  (DMA + PE transpose + matmul per block) before the edge pass.
- W1ext is 260 cols (h|s) instead of 264: the src-side d column was never
  read.
- Per-block tile counts (max over cores) instead of one global max: ~5%
  fewer edge tiles (gathers, matmuls, DVE work).
- L2 output accumulation: tensor_reduce writes the out_all column directly;
  the +cfc constant is applied once at the end (v2 paid a slow read-modify
  -write tensor_scalar per block).
"""
import sys
from contextlib import ExitStack
import numpy as np
import ml_dtypes

sys.path.insert(0, '/opt/trn_rl_repo')
sys.path.insert(0, '/root/problem')

# ---------------- problem constants (hardcoded from the spec) --------------
N = 50000
E = 800000
FIN = 128
H = 4
C1 = 64
C2 = 32
D1 = H * C1            # 256
D2 = H * C2            # 128
R1 = D1 + H            # 260 W1ext cols: h(256)|s(4)
R2 = D2 + 2 * H        # 136 W2ext cols: h2(128)|s2(4)|d2(4)
NCORES = 8
NO = N // NCORES       # 6250 owned dst nodes / core
P = 128
NB = 49                # dst blocks per core
NOP = NB * P           # padded own nodes (6272)
NROWS = NCORES * NOP   # 50176 table rows (row = owner*NOP + local)
SPLIT = 32768          # int16 gather index limit
EPS_BN = 1e-5
GNT = 7                # gather tiles per dma_gather call

BF = ml_dtypes.bfloat16


def _bf(a):
    return np.asarray(a, np.float32).astype(BF)


# ---------------------------- host preprocessing ---------------------------
def preprocess(x, edge_index, W1, a1_src, a1_dst, b1, g1, be1, m1, v1,
               W2, a2_src, a2_dst, b2, g2, be2, m2, v2, fcW, fcb):
    x = np.asarray(x, np.float32)
    ei = np.asarray(edge_index)
    src = np.concatenate([ei[0], np.arange(N, dtype=np.int64)]).astype(np.int64)
    dst = np.concatenate([ei[1], np.arange(N, dtype=np.int64)]).astype(np.int64)

    W1 = np.asarray(W1, np.float32); W2 = np.asarray(W2, np.float32)
    a1_src = np.asarray(a1_src, np.float32); a1_dst = np.asarray(a1_dst, np.float32)
    a2_src = np.asarray(a2_src, np.float32); a2_dst = np.asarray(a2_dst, np.float32)
    g1 = np.asarray(g1, np.float32); be1 = np.asarray(be1, np.float32)
    m1 = np.asarray(m1, np.float32); v1 = np.asarray(v1, np.float32)
    g2 = np.asarray(g2, np.float32); be2 = np.asarray(be2, np.float32)
    m2 = np.asarray(m2, np.float32); v2 = np.asarray(v2, np.float32)
    b1 = np.asarray(b1, np.float32); b2 = np.asarray(b2, np.float32)
    fcW = np.asarray(fcW, np.float32); fcb = np.asarray(fcb, np.float32)

    scale1 = g1 / np.sqrt(v1 + EPS_BN)
    shtot1 = scale1 * b1 + (be1 - m1 * scale1)                    # [256]
    W1p = W1 * scale1[None, :]
    w_s1 = np.einsum('fhc,hc->fh', W1.reshape(FIN, H, C1), a1_src)
    w_d1 = np.einsum('fhc,hc->fh', W1.reshape(FIN, H, C1), a1_dst)
    W1ext = np.concatenate([W1p, w_s1], axis=1)                   # [128,260]

    scale2 = g2 / np.sqrt(v2 + EPS_BN)
    shtot2 = scale2 * b2 + (be2 - m2 * scale2)                    # [128]
    W2p = W2 * scale2[None, :]
    w_s2 = np.einsum('fhc,hc->fh', W2.reshape(D1, H, C2), a2_src)
    w_d2 = np.einsum('fhc,hc->fh', W2.reshape(D1, H, C2), a2_dst)
    W2ext = np.concatenate([W2p, w_s2, w_d2], axis=1)             # [256,136]
    # ELU(-1) fold must be consistent with the QUANTIZED weights the device
    # matmul actually uses, else each column picks up a systematic bias.
    c2 = -(_bf(W2ext).astype(np.float32)).sum(axis=0).astype(np.float32)
    cfc = float(fcb[0] - fcW.sum())
    fcw_row = fcW.reshape(D2).astype(np.float32)

    # d1 per own node from bf16-quantized x/w_d1 (tracks the device matmul)
    xq = _bf(x).astype(np.float32)
    wdq = _bf(w_d1).astype(np.float32)
    d1_full = xq @ wdq                                            # [N, H]

    # shared table row id for both layers
    rows_all = ((src // NO) * NOP + (src % NO)).astype(np.int32)

    # --- per-core edge routing; per-block tile counts (max over cores) ---
    owner = dst // NO
    per_core = []
    nA = np.zeros((NCORES, NB), np.int64)
    nBn = np.zeros((NCORES, NB), np.int64)
    for c in range(NCORES):
        m = owner == c
        r_c = rows_all[m]
        dl = (dst[m] - c * NO).astype(np.int64)
        order = np.argsort(dl, kind='stable')
        r_c = r_c[order]; dl = dl[order]
        blk = dl // P
        cnt = np.bincount(blk, minlength=NB)
        blocks = []
        start = 0
        for b in range(NB):
            n_b = int(cnt[b])
            sl = slice(start, start + n_b)
            r = r_c[sl]; d = (dl[sl] - b * P).astype(np.int64)
            isa = r < SPLIT
            blocks.append((r[isa], d[isa], r[~isa], d[~isa]))
            nA[c, b] = len(blocks[-1][0])
            nBn[c, b] = len(blocks[-1][2])
            start += n_b
        per_core.append(blocks)
    TAb = np.maximum(1, -(-nA.max(axis=0) // P)).astype(np.int64)   # [NB]
    TBb = np.maximum(1, -(-nBn.max(axis=0) // P)).astype(np.int64)  # [NB]
    Tb = (TAb + TBb).astype(np.int64)
    toff = np.concatenate([[0], np.cumsum(Tb)]).astype(np.int64)    # [NB+1]
    Ttot = int(toff[-1])

    def wrap16(a):  # flat [K] int16 -> [16, K//16]: w[p, s] = a[s*16+p]
        return np.ascontiguousarray(a.reshape(-1, 16).T).astype(np.int16)

    in_maps = []
    layout = None
    for c in range(NCORES):
        gidx = np.zeros((Ttot, P), np.int16)     # x / t2 row gather indices
        dloc = np.full((Ttot, P), 255.0, np.float32)
        for b in range(NB):
            ra, da, rb, db = per_core[c][b]
            o = int(toff[b])
            na, nb_ = len(ra), len(rb)
            gidx[o:o + TAb[b]].reshape(-1)[:na] = ra.astype(np.int16)
            dloc[o:o + TAb[b]].reshape(-1)[:na] = da
            ob = int(toff[b] + TAb[b])
            gidx[ob:ob + TBb[b]].reshape(-1)[:nb_] = (rb - SPLIT).astype(np.int16)
            dloc[ob:ob + TBb[b]].reshape(-1)[:nb_] = db
        gw = wrap16(gidx.reshape(-1))                       # [16, Ttot*8]
        dlocP = np.ascontiguousarray(                        # [128, Ttot] u8
            dloc.reshape(Ttot, P).T).astype(np.uint8)

        xs = np.zeros((NOP, FIN), BF)
        xs[:NO] = _bf(x[c * NO:(c + 1) * NO])

        d1a = np.zeros((NOP, H), np.float32)
        d1a[:NO] = d1_full[c * NO:(c + 1) * NO]
        # [128, NB*H]: d1aP[p, b*H+h] = d1 of node b*128+p
        d1aP = np.ascontiguousarray(
            d1a.reshape(NB, P, H).transpose(1, 0, 2).reshape(P, NB * H)
        ).astype(BF)

        sections = [
            ("xs", xs),                                    # [NOP,128] bf16
            ("gidx", gw),                                  # [16, Ttot*8] i16
            ("dloc", dlocP),                               # [128, Ttot] u8
            ("d1a", d1aP),                                 # [128, NB*H] bf16
            ("w1e", _bf(W1ext)),
            ("w2e", _bf(np.concatenate([W2ext[:P], W2ext[P:]], axis=1))),
            ("sh1", np.tile(shtot1.astype(np.float32).reshape(1, -1), (P, 1))),
            ("sh2", np.tile(shtot2.astype(np.float32).reshape(1, -1), (P, 1))),
            ("c2", np.tile(c2.reshape(1, -1), (P, 1))),
            ("fcw", np.tile(fcw_row.reshape(1, -1), (P, 1))),
        ]
        offs = {}
        cur = 0
        bufs = []
        for name, arr in sections:
            bb = np.ascontiguousarray(arr).view(np.uint8).reshape(arr.shape[0], -1)
            offs[name] = (cur, bb.shape[0], bb.shape[1])
            bufs.append(bb.reshape(-1))
            cur += bb.size
            pad = (-cur) % 64
            if pad:
                bufs.append(np.zeros(pad, np.uint8))
                cur += pad
        blob = np.concatenate(bufs)
        if layout is None:
            layout = offs
            nbytes = len(blob)
        in_maps.append({"blob": blob.reshape(1, -1)})
    meta = (TAb.tolist(), TBb.tolist(), toff.tolist(), Ttot)
    return in_maps, meta, layout, nbytes, cfc


# ------------------------------ bass builder -------------------------------
def build_module(meta, layout, nbytes, cfc):
    from concourse import bass, mybir, bacc
    import concourse.tile as tile
    from concourse.masks import make_identity

    TAb, TBb, toff, Ttot = meta
    f32 = mybir.dt.float32
    bf16 = mybir.dt.bfloat16
    i16 = mybir.dt.int16
    u8 = mybir.dt.uint8
    AF = mybir.ActivationFunctionType
    OP = mybir.AluOpType

    nc = bacc.Bacc(dynamic_dma_scratch_size=65536, num_swdge_queues=4)
    blob_p = nc.declare_dram_parameter("blob", [1, nbytes], u8, isOutput=False)
    out_p = nc.declare_dram_parameter("out", [P, NB], f32, isOutput=True)

    xfull = nc.dram_tensor("xfull", [NROWS, 2 * FIN], u8, addr_space="Shared")
    t2own = nc.dram_tensor("t2own", [NOP, 512], u8)
    t2full = nc.dram_tensor("t2full", [NROWS, 512], u8, addr_space="Shared")
    d2sc = nc.dram_tensor("d2sc", [P, NB * H], bf16)

    blob_ap = blob_p[:]

    def bv(name, dt):
        off, parts, wbytes = layout[name]
        ap = bass.AP(tensor=blob_ap.tensor, offset=off,
                     ap=[[wbytes, parts], [1, wbytes]])
        return ap.bitcast(dt)

    def apx(base_ap, off, pattern):
        return bass.AP(tensor=base_ap.tensor, offset=base_ap.offset + off,
                       ap=[list(base_ap.ap[0])] + [list(q) for q in pattern])

    o_xs = layout["xs"][0]

    # ---------------- Block 0: AllGather x stripes -------------------------
    xs_ap = bass.AP(tensor=blob_ap.tensor, offset=o_xs,
                    ap=[[2 * FIN, NOP], [1, 2 * FIN]])
    xsown = nc.dram_tensor("xsown", [NOP, 2 * FIN], u8)
    with (nc.Block() as block, nc.semaphore("ccx") as ccx,
          nc.semaphore("xcp") as xcp):
        @block.gpsimd
        def _(gp):
            gp.dma_start(out=xsown[:], in_=xs_ap).then_inc(xcp, 16)
            gp.wait_ge(xcp, 16)
            gp.collective_compute(
                "AllGather", mybir.AluOpType.bypass,
                replica_groups=[list(range(NCORES))],
                ins=[xsown[:]], outs=[xfull[:]],
            ).then_inc(ccx)
            gp.wait_ge(ccx, 1)

    def load_consts(tc, ctxpool):
        idx_t = ctxpool.tile([P, Ttot * 8], i16, tag="idx")
        for k in range(8):
            nc.sync.dma_start(out=idx_t[16 * k:16 * (k + 1), :],
                              in_=bv("gidx", i16))
        dloc8 = ctxpool.tile([P, Ttot], u8, tag="dloc8")
        nc.sync.dma_start(out=dloc8[:], in_=bv("dloc", u8))
        dloc_t = ctxpool.tile([P, Ttot], bf16, tag="dloc")
        nc.vector.tensor_copy(dloc_t[:], dloc8[:])
        ioti = ctxpool.tile([P, P], mybir.dt.int32, tag="ioti")
        nc.gpsimd.iota(ioti[:], pattern=[[1, P]], base=0, channel_multiplier=0)
        iotab = ctxpool.tile([P, P], bf16, tag="iotab")
        nc.vector.tensor_copy(iotab[:], ioti[:])
        identb = ctxpool.tile([P, P], bf16, tag="identb")
        make_identity(nc, identb[:])
        return idx_t, dloc_t, iotab, identb

    def gathers(dest_ap_fn, idx_src, goff, ntiles, in_ap, elem, transpose,
                qn=0):
        done = 0
        while done < ntiles:
            nt = min(ntiles - done, GNT)
            K = nt * P
            col0 = (goff + done) * 8
            nc.gpsimd.dma_gather(
                out_ap=dest_ap_fn(done, nt),
                in_ap=in_ap,
                idxs_ap=idx_src[:, col0:col0 + nt * 8],
                num_idxs=K, num_idxs_reg=K, elem_size=elem,
                transpose=transpose, queue_num=qn)
            done += nt

    # ---------------- ctx 1: layer-1 edge pass -----------------------------
    with tile.TileContext(nc) as tc, ExitStack() as ctx:
        consts = ctx.enter_context(tc.tile_pool(name="c1", bufs=1))
        dpool = ctx.enter_context(tc.tile_pool(name="dp1", bufs=1))
        sb = ctx.enter_context(tc.tile_pool(name="s1", bufs=4))
        sb2 = ctx.enter_context(tc.tile_pool(name="s1b", bufs=2))
        pstp = ctx.enter_context(tc.tile_pool(name="ptp", bufs=2, space="PSUM"))
        psbig = ctx.enter_context(tc.tile_pool(name="pbig", bufs=4,
                                               space="PSUM"))
        pspo = ctx.enter_context(tc.tile_pool(name="ppo", bufs=2, space="PSUM"))

        idx_t, dloc_t, iotab, identb = load_consts(tc, consts)
        w1e = consts.tile([P, R1], bf16, tag="w1e")
        nc.sync.dma_start(out=w1e[:], in_=bv("w1e", bf16))
        w2e = consts.tile([P, 2, R2], bf16, tag="w2e")
        nc.sync.dma_start(out=w2e[:], in_=bv("w2e", bf16))
        sh1r = consts.tile([P, D1], f32, tag="sh1")
        nc.sync.dma_start(out=sh1r[:], in_=bv("sh1", f32))
        c2r = consts.tile([P, R2], f32, tag="c2")
        nc.sync.dma_start(out=c2r[:], in_=bv("c2", f32))
        d1a = consts.tile([P, NB * H], bf16, tag="d1a")
        nc.sync.dma_start(out=d1a[:], in_=bv("d1a", bf16))

        d2a = dpool.tile([P, NB * H], bf16, tag="d2a")

        xfull_bf = xfull[:].bitcast(bf16)
        xfullB_bf = xfull[SPLIT:NROWS, :].bitcast(bf16)

        for b in range(NB):
            TA = int(TAb[b]); TB = int(TBb[b])
            T = TA + TB
            o = int(toff[b])
            xTgA = sb.tile([P, TA * P], bf16, tag="xTgA")
            xTgB = sb.tile([P, TB * P], bf16, tag="xTgB")

            def xdstA(t0, nt, _x=xTgA):
                return apx(_x[:], t0 * P, [[1, 1], [1, nt * P]])

            def xdstB(t0, nt, _x=xTgB):
                return apx(_x[:], t0 * P, [[1, 1], [1, nt * P]])

            gathers(xdstA, idx_t, o, TA, xfull_bf, FIN, True, qn=b % 4)
            gathers(xdstB, idx_t, o + TA, TB, xfullB_bf, FIN, True,
                    qn=(b + 2) % 4)

            me = sb.tile([P, T * P], bf16, tag="me")
            nc.vector.tensor_tensor(
                out=apx(me[:], 0, [[P, T], [1, P]]),
                in0=apx(dloc_t[:], o, [[1, T], [0, P]]),
                in1=apx(iotab[:], 0, [[0, T], [1, P]]),
                op=OP.is_equal)

            G_all = sb.tile([P, T * R1], bf16, tag="G_all")
            po = pspo.tile([P, R1], f32, tag="po")
            meTa = sb.tile([P, T * P], bf16, tag="meTa")
            for t in range(T):
                tp = pstp.tile([P, P], bf16, tag="tp")
                nc.tensor.transpose(out=tp[:], in_=me[:, t * P:(t + 1) * P],
                                    identity=identb[:])
                nc.vector.tensor_copy(meTa[:, t * P:(t + 1) * P], tp[:])
            for t in range(T):
                Gps = psbig.tile([P, R1], f32, tag="Gps")
                xsrc = (xTgA[:, t * P:(t + 1) * P] if t < TA
                        else xTgB[:, (t - TA) * P:(t - TA + 1) * P])
                nc.tensor.matmul(out=Gps[:], lhsT=xsrc,
                                 rhs=w1e[:], start=True, stop=False)
                nc.tensor.matmul(out=Gps[:, D1:R1],
                                 lhsT=meTa[:, t * P:(t + 1) * P],
                                 rhs=d1a[:, b * H:(b + 1) * H],
                                 start=False, stop=True)
                nc.vector.tensor_copy(
                    G_all[:, t * R1:(t + 1) * R1], Gps[:])

            e1 = sb2.tile([P, T * H], bf16, tag="e1")
            nc.scalar.activation(e1[:], apx(G_all[:], D1, [[R1, T], [1, H]]),
                                 AF.Exp)
            e2 = sb2.tile([P, T * H], bf16, tag="e2")
            nc.scalar.activation(e2[:], apx(G_all[:], D1, [[R1, T], [1, H]]),
                                 AF.Exp, scale=0.2)
            ex = sb2.tile([P, T * H], bf16, tag="ex")
            nc.vector.tensor_tensor(out=ex[:], in0=e1[:], in1=e2[:], op=OP.max)
            we = sb2.tile([P, T * R1], bf16, tag="we")
            nc.vector.tensor_tensor(
                out=apx(we[:], 0, [[R1, T], [1, D1]]),
                in0=apx(G_all[:], 0, [[R1, T], [1, D1]]),
                in1=apx(ex[:], 0, [[H, T], [1, H], [0, C1]]), op=OP.mult)
            nc.vector.tensor_copy(
                out=apx(we[:], D1, [[R1, T], [1, H]]),
                in_=apx(ex[:], 0, [[H, T], [1, H]]))
            for t in range(T):
                nc.tensor.matmul(
                    out=po[:], lhsT=me[:, t * P:(t + 1) * P],
                    rhs=we[:, t * R1:(t + 1) * R1],
                    start=(t == 0), stop=(t == T - 1))

            den = sb2.tile([P, H], f32, tag="den")
            nc.vector.tensor_scalar_max(den[:], po[:, D1:R1], 1e-30)
            rden = sb2.tile([P, H], f32, tag="rden")
            nc.vector.reciprocal(rden[:], den[:])
            r = sb2.tile([P, D1], f32, tag="r")
            nc.vector.tensor_tensor(
                out=apx(r[:], 0, [[C1, H], [1, C1]]),
                in0=apx(po[:], 0, [[C1, H], [1, C1]]),
                in1=apx(rden[:], 0, [[1, H], [0, C1]]), op=OP.mult)
            nc.vector.tensor_add(r[:], r[:], sh1r[:])
            tneg = sb2.tile([P, D1], f32, tag="tneg")
            nc.vector.tensor_scalar_min(tneg[:], r[:], 0.0)
            texp = sb2.tile([P, D1], f32, tag="texp")
            nc.scalar.activation(texp[:], tneg[:], AF.Exp)
            ub = sb2.tile([P, D1], f32, tag="ub")
            nc.vector.tensor_scalar_max(ub[:], r[:], 0.0)
            ubb = sb2.tile([P, D1], bf16, tag="ubb")
            nc.vector.tensor_tensor(out=ubb[:], in0=ub[:], in1=texp[:],
                                    op=OP.add)
            uT = sb2.tile([P, 2, P], bf16, tag="uT")
            for k in range(2):
                tp = pstp.tile([P, P], bf16, tag="tp")
                nc.tensor.transpose(out=tp[:], in_=ubb[:, k * P:(k + 1) * P],
                                    identity=identb[:])
                nc.vector.tensor_copy(uT[:, k, :], tp[:])
            p2 = psbig.tile([P, R2], f32, tag="Gps")
            for k in range(2):
                nc.tensor.matmul(out=p2[:], lhsT=uT[:, k, :], rhs=w2e[:, k, :],
                                 start=(k == 0), stop=(k == 1))
            row2 = sb2.tile([P, R2], bf16, tag="row2")
            nc.vector.tensor_tensor(out=row2[:], in0=p2[:], in1=c2r[:],
                                    op=OP.add)
            nc.vector.tensor_copy(d2a[:, b * H:(b + 1) * H],
                                  row2[:, D2 + H:R2])
            t2o_ap = bass.AP(tensor=t2own[:].tensor, offset=b * P * 512,
                             ap=[[512, P], [1, 2 * R2]]).bitcast(bf16)
            nc.sync.dma_start(out=t2o_ap, in_=row2[:])

        nc.sync.dma_start(out=d2sc[:], in_=d2a[:])

    # ---------------- Block 1: AllGather t2 rows ---------------------------
    with (nc.Block() as block, nc.semaphore("cct") as cct):
        @block.gpsimd
        def _(gp):
            gp.collective_compute(
                "AllGather", mybir.AluOpType.bypass,
                replica_groups=[list(range(NCORES))],
                ins=[t2own[:]], outs=[t2full[:]],
            ).then_inc(cct)
            gp.wait_ge(cct, 1)

    # ---------------- ctx 2: layer-2 edge pass -----------------------------
    with tile.TileContext(nc) as tc, ExitStack() as ctx:
        consts = ctx.enter_context(tc.tile_pool(name="c2p", bufs=1))
        dpool = ctx.enter_context(tc.tile_pool(name="dp2", bufs=1))
        sb = ctx.enter_context(tc.tile_pool(name="s2", bufs=5))
        sb2 = ctx.enter_context(tc.tile_pool(name="s2b", bufs=3))
        pstp = ctx.enter_context(tc.tile_pool(name="ptp2", bufs=4, space="PSUM"))
        psz = ctx.enter_context(tc.tile_pool(name="psz", bufs=2, space="PSUM"))
        pspo = ctx.enter_context(tc.tile_pool(name="ppo2", bufs=2, space="PSUM"))

        idx_t, dloc_t, iotab, identb = load_consts(tc, consts)
        sh2r = consts.tile([P, D2], f32, tag="sh2")
        nc.sync.dma_start(out=sh2r[:], in_=bv("sh2", f32))
        fcwr = consts.tile([P, D2], f32, tag="fcw")
        nc.sync.dma_start(out=fcwr[:], in_=bv("fcw", f32))
        d2a = consts.tile([P, NB * H], bf16, tag="d2a")
        nc.sync.dma_start(out=d2a[:], in_=d2sc[:])
        out_all = dpool.tile([P, NB], f32, tag="out_all")

        t2_bf = t2full[:].bitcast(bf16)
        t2B_bf = t2full[SPLIT:NROWS, :].bitcast(bf16)

        for b in range(NB):
            TA = int(TAb[b]); TB = int(TBb[b])
            T = TA + TB
            o = int(toff[b])
            G2 = sb.tile([P, T, 256], bf16, tag="G2")

            def gdst(t0, nt, _g=G2):
                return _g[:, t0:t0 + nt, :]

            def gdstB(t0, nt, _g=G2, _ta=TA):
                return _g[:, _ta + t0:_ta + t0 + nt, :]

            gathers(gdst, idx_t, o, TA, t2_bf, 256, False, qn=b % 4)
            gathers(gdstB, idx_t, o + TA, TB, t2B_bf, 256, False, qn=b % 4)

            me = sb.tile([P, T * P], bf16, tag="me2")
            nc.vector.tensor_tensor(
                out=apx(me[:], 0, [[P, T], [1, P]]),
                in0=apx(dloc_t[:], o, [[1, T], [0, P]]),
                in1=apx(iotab[:], 0, [[0, T], [1, P]]),
                op=OP.is_equal)

            z2 = sb2.tile([P, T * H], bf16, tag="z2")
            po2 = pspo.tile([P, R2 - H], f32, tag="po2")
            meTa = sb.tile([P, T * P], bf16, tag="meTa2")
            for t in range(T):
                tp = pstp.tile([P, P], bf16, tag="tp")
                nc.tensor.transpose(out=tp[:], in_=me[:, t * P:(t + 1) * P],
                                    identity=identb[:])
                nc.vector.tensor_copy(meTa[:, t * P:(t + 1) * P], tp[:])
            zps = psz.tile([P, T * H], f32, tag="zps")
            for t in range(T):
                nc.tensor.matmul(out=zps[:, t * H:(t + 1) * H],
                                 lhsT=meTa[:, t * P:(t + 1) * P],
                                 rhs=d2a[:, b * H:(b + 1) * H],
                                 start=True, stop=True)
            nc.vector.tensor_tensor(
                out=z2[:], in0=zps[:],
                in1=apx(G2[:], D2, [[256, T], [1, H]]), op=OP.add)

            e1 = sb2.tile([P, T * H], bf16, tag="e1b")
            nc.scalar.activation(e1[:], z2[:], AF.Exp)
            e2 = sb2.tile([P, T * H], bf16, tag="e2b")
            nc.scalar.activation(e2[:], z2[:], AF.Exp, scale=0.2)
            ex = sb2.tile([P, T * H], bf16, tag="ex2")
            nc.vector.tensor_tensor(out=ex[:], in0=e1[:], in1=e2[:], op=OP.max)
            we = sb2.tile([P, T * (R2 - H)], bf16, tag="we2")
            nc.vector.tensor_tensor(
                out=apx(we[:], 0, [[R2 - H, T], [1, D2]]),
                in0=apx(G2[:], 0, [[256, T], [1, D2]]),
                in1=apx(ex[:], 0, [[H, T], [1, H], [0, C2]]), op=OP.mult)
            nc.vector.tensor_copy(
                out=apx(we[:], D2, [[R2 - H, T], [1, H]]),
                in_=apx(ex[:], 0, [[H, T], [1, H]]))
            for t in range(T):
                nc.tensor.matmul(
                    out=po2[:], lhsT=me[:, t * P:(t + 1) * P],
                    rhs=we[:, t * (R2 - H):(t + 1) * (R2 - H)],
                    start=(t == 0), stop=(t == T - 1))

            den = sb2.tile([P, H], f32, tag="den2")
            nc.vector.tensor_scalar_max(den[:], po2[:, D2:D2 + H], 1e-30)
            rden = sb2.tile([P, H], f32, tag="rden2")
            nc.vector.reciprocal(rden[:], den[:])
            r = sb2.tile([P, D2], f32, tag="r2")
            nc.vector.tensor_tensor(
                out=apx(r[:], 0, [[C2, H], [1, C2]]),
                in0=apx(po2[:], 0, [[C2, H], [1, C2]]),
                in1=apx(rden[:], 0, [[1, H], [0, C2]]), op=OP.mult)
            nc.vector.tensor_add(r[:], r[:], sh2r[:])
            tneg = sb2.tile([P, D2], f32, tag="tneg2")
            nc.vector.tensor_scalar_min(tneg[:], r[:], 0.0)
            texp = sb2.tile([P, D2], f32, tag="texp2")
            nc.scalar.activation(texp[:], tneg[:], AF.Exp)
            u2 = sb2.tile([P, D2], f32, tag="u2")
            nc.vector.tensor_scalar_max(u2[:], r[:], 0.0)
            nc.vector.tensor_add(u2[:], u2[:], texp[:])
            prodt = sb2.tile([P, D2], f32, tag="prodt")
            nc.vector.tensor_tensor(out=prodt[:], in0=u2[:], in1=fcwr[:],
                                    op=OP.mult)
            nc.vector.tensor_reduce(out_all[:, b:b + 1], prodt[:],
                                    axis=mybir.AxisListType.X, op=OP.add)

        outf = dpool.tile([P, NB], f32, tag="outf")
        nc.vector.tensor_scalar_add(outf[:], out_all[:], float(cfc))
        nc.sync.dma_start(out=out_p[:], in_=outf[:])

    nc.finalize()
    return nc


def run_spmd(nc, in_maps):
    from concourse.bass_utils import run_bass_kernel_spmd
    res = run_bass_kernel_spmd(nc, in_maps, core_ids=list(range(NCORES)))
    return res.results


def kernel(**inputs):
    in_maps, meta, layout, nbytes, cfc = preprocess(**inputs)
    nc = build_module(meta, layout, nbytes, cfc)
    results = run_spmd(nc, in_maps)
    out = np.empty((N, 1), np.float32)
    for c in range(NCORES):
        o = results[c]["out"]                 # [128, 49] partition-major
        y = np.ascontiguousarray(o.T).reshape(NOP)
        out[c * NO:(c + 1) * NO, 0] = y[:NO]
    return out


# revision 26
# speedup vs baseline: 1.0204x; 1.0204x over previous
"""Distributed 2-layer GAT (BangaloreGAT) on 8 TRN2 NeuronCores — v4.

v2 baseline plus low-risk wins (engine/sync patterns kept identical to v2,
which is stable across hundreds of executions; more aggressive restructures
showed intermittent DMA/engine races on this stack):
- d1 (layer-1 dst attention term per own node) is computed on the host and
  shipped in the blob (50 KB), removing v2's 49-iteration device prologue
  (DMA + PE transpose + matmul per block) before the edge pass.
- W1ext is 260 cols (h|s) instead of 264: the src-side d column was never
  read.
- Per-block tile counts (max over cores) instead of one global max: ~5%
  fewer edge tiles (gathers, matmuls, DVE work).
- L2 output accumulation: tensor_reduce writes the out_all column directly;
  the +cfc constant is applied once at the end (v2 paid a slow read-modify
  -write tensor_scalar per block).
"""
import sys
from contextlib import ExitStack
import numpy as np
import ml_dtypes

sys.path.insert(0, '/opt/trn_rl_repo')
sys.path.insert(0, '/root/problem')

# ---------------- problem constants (hardcoded from the spec) --------------
N = 50000
E = 800000
FIN = 128
H = 4
C1 = 64
C2 = 32
D1 = H * C1            # 256
D2 = H * C2            # 128
R1 = D1 + H            # 260 W1ext cols: h(256)|s(4)
R2 = D2 + 2 * H        # 136 W2ext cols: h2(128)|s2(4)|d2(4)
NCORES = 8
NO = N // NCORES       # 6250 owned dst nodes / core
P = 128
NB = 49                # dst blocks per core
NOP = NB * P           # padded own nodes (6272)
NROWS = NCORES * NOP   # 50176 table rows (row = owner*NOP + local)
SPLIT = 32768          # int16 gather index limit
EPS_BN = 1e-5
GNT = 7                # gather tiles per dma_gather call

BF = ml_dtypes.bfloat16


def _bf(a):
    return np.asarray(a, np.float32).astype(BF)


# ---------------------------- host preprocessing ---------------------------
def preprocess(x, edge_index, W1, a1_src, a1_dst, b1, g1, be1, m1, v1,
               W2, a2_src, a2_dst, b2, g2, be2, m2, v2, fcW, fcb):
    x = np.asarray(x, np.float32)
    ei = np.asarray(edge_index)
    src = np.concatenate([ei[0], np.arange(N, dtype=np.int64)]).astype(np.int64)
    dst = np.concatenate([ei[1], np.arange(N, dtype=np.int64)]).astype(np.int64)

    W1 = np.asarray(W1, np.float32); W2 = np.asarray(W2, np.float32)
    a1_src = np.asarray(a1_src, np.float32); a1_dst = np.asarray(a1_dst, np.float32)
    a2_src = np.asarray(a2_src, np.float32); a2_dst = np.asarray(a2_dst, np.float32)
    g1 = np.asarray(g1, np.float32); be1 = np.asarray(be1, np.float32)
    m1 = np.asarray(m1, np.float32); v1 = np.asarray(v1, np.float32)
    g2 = np.asarray(g2, np.float32); be2 = np.asarray(be2, np.float32)
    m2 = np.asarray(m2, np.float32); v2 = np.asarray(v2, np.float32)
    b1 = np.asarray(b1, np.float32); b2 = np.asarray(b2, np.float32)
    fcW = np.asarray(fcW, np.float32); fcb = np.asarray(fcb, np.float32)

    scale1 = g1 / np.sqrt(v1 + EPS_BN)
    shtot1 = scale1 * b1 + (be1 - m1 * scale1)                    # [256]
    W1p = W1 * scale1[None, :]
    w_s1 = np.einsum('fhc,hc->fh', W1.reshape(FIN, H, C1), a1_src)
    w_d1 = np.einsum('fhc,hc->fh', W1.reshape(FIN, H, C1), a1_dst)
    W1ext = np.concatenate([W1p, w_s1], axis=1)                   # [128,260]

    scale2 = g2 / np.sqrt(v2 + EPS_BN)
    shtot2 = scale2 * b2 + (be2 - m2 * scale2)                    # [128]
    W2p = W2 * scale2[None, :]
    w_s2 = np.einsum('fhc,hc->fh', W2.reshape(D1, H, C2), a2_src)
    w_d2 = np.einsum('fhc,hc->fh', W2.reshape(D1, H, C2), a2_dst)
    W2ext = np.concatenate([W2p, w_s2, w_d2], axis=1)             # [256,136]
    # ELU(-1) fold must be consistent with the QUANTIZED weights the device
    # matmul actually uses, else each column picks up a systematic bias.
    c2 = -(_bf(W2ext).astype(np.float32)).sum(axis=0).astype(np.float32)
    cfc = float(fcb[0] - fcW.sum())
    fcw_row = fcW.reshape(D2).astype(np.float32)

    # d1 per own node from bf16-quantized x/w_d1 (tracks the device matmul)
    xq = _bf(x).astype(np.float32)
    wdq = _bf(w_d1).astype(np.float32)
    d1_full = xq @ wdq                                            # [N, H]

    # shared table row id for both layers
    rows_all = ((src // NO) * NOP + (src % NO)).astype(np.int32)

    # --- per-core edge routing; per-block tile counts (max over cores) ---
    owner = dst // NO
    per_core = []
    nA = np.zeros((NCORES, NB), np.int64)
    nBn = np.zeros((NCORES, NB), np.int64)
    for c in range(NCORES):
        m = owner == c
        r_c = rows_all[m]
        dl = (dst[m] - c * NO).astype(np.int64)
        order = np.argsort(dl, kind='stable')
        r_c = r_c[order]; dl = dl[order]
        blk = dl // P
        cnt = np.bincount(blk, minlength=NB)
        blocks = []
        start = 0
        for b in range(NB):
            n_b = int(cnt[b])
            sl = slice(start, start + n_b)
            r = r_c[sl]; d = (dl[sl] - b * P).astype(np.int64)
            isa = r < SPLIT
            blocks.append((r[isa], d[isa], r[~isa], d[~isa]))
            nA[c, b] = len(blocks[-1][0])
            nBn[c, b] = len(blocks[-1][2])
            start += n_b
        per_core.append(blocks)
    TAb = np.maximum(1, -(-nA.max(axis=0) // P)).astype(np.int64)   # [NB]
    TBb = np.maximum(1, -(-nBn.max(axis=0) // P)).astype(np.int64)  # [NB]
    Tb = (TAb + TBb).astype(np.int64)
    toff = np.concatenate([[0], np.cumsum(Tb)]).astype(np.int64)    # [NB+1]
    Ttot = int(toff[-1])

    def wrap16(a):  # flat [K] int16 -> [16, K//16]: w[p, s] = a[s*16+p]
        return np.ascontiguousarray(a.reshape(-1, 16).T).astype(np.int16)

    in_maps = []
    layout = None
    for c in range(NCORES):
        gidx = np.zeros((Ttot, P), np.int16)     # x / t2 row gather indices
        dloc = np.full((Ttot, P), 255.0, np.float32)
        for b in range(NB):
            ra, da, rb, db = per_core[c][b]
            o = int(toff[b])
            na, nb_ = len(ra), len(rb)
            gidx[o:o + TAb[b]].reshape(-1)[:na] = ra.astype(np.int16)
            dloc[o:o + TAb[b]].reshape(-1)[:na] = da
            ob = int(toff[b] + TAb[b])
            gidx[ob:ob + TBb[b]].reshape(-1)[:nb_] = (rb - SPLIT).astype(np.int16)
            dloc[ob:ob + TBb[b]].reshape(-1)[:nb_] = db
        gw = wrap16(gidx.reshape(-1))                       # [16, Ttot*8]
        dlocP = np.ascontiguousarray(                        # [128, Ttot] u8
            dloc.reshape(Ttot, P).T).astype(np.uint8)

        xs = np.zeros((NOP, FIN), BF)
        xs[:NO] = _bf(x[c * NO:(c + 1) * NO])

        d1a = np.zeros((NOP, H), np.float32)
        d1a[:NO] = d1_full[c * NO:(c + 1) * NO]
        # [128, NB*H]: d1aP[p, b*H+h] = d1 of node b*128+p
        d1aP = np.ascontiguousarray(
            d1a.reshape(NB, P, H).transpose(1, 0, 2).reshape(P, NB * H)
        ).astype(BF)

        sections = [
            ("xs", xs),                                    # [NOP,128] bf16
            ("gidx", gw),                                  # [16, Ttot*8] i16
            ("dloc", dlocP),                               # [128, Ttot] u8
            ("d1a", d1aP),                                 # [128, NB*H] bf16
            ("w1e", _bf(W1ext)),
            ("w2e", _bf(np.concatenate([W2ext[:P], W2ext[P:]], axis=1))),
            ("sh1", np.tile(shtot1.astype(np.float32).reshape(1, -1), (P, 1))),
            ("sh2", np.tile(shtot2.astype(np.float32).reshape(1, -1), (P, 1))),
            ("c2", np.tile(c2.reshape(1, -1), (P, 1))),
            ("fcw", np.tile(fcw_row.reshape(1, -1), (P, 1))),
        ]
        offs = {}
        cur = 0
        bufs = []
        for name, arr in sections:
            bb = np.ascontiguousarray(arr).view(np.uint8).reshape(arr.shape[0], -1)
            offs[name] = (cur, bb.shape[0], bb.shape[1])
            bufs.append(bb.reshape(-1))
            cur += bb.size
            pad = (-cur) % 64
            if pad:
                bufs.append(np.zeros(pad, np.uint8))
                cur += pad
        blob = np.concatenate(bufs)
        if layout is None:
            layout = offs
            nbytes = len(blob)
        in_maps.append({"blob": blob.reshape(1, -1)})
    meta = (TAb.tolist(), TBb.tolist(), toff.tolist(), Ttot)
    return in_maps, meta, layout, nbytes, cfc


# ------------------------------ bass builder -------------------------------
def build_module(meta, layout, nbytes, cfc):
    from concourse import bass, mybir, bacc
    import concourse.tile as tile
    from concourse.masks import make_identity

    TAb, TBb, toff, Ttot = meta
    f32 = mybir.dt.float32
    bf16 = mybir.dt.bfloat16
    i16 = mybir.dt.int16
    u8 = mybir.dt.uint8
    AF = mybir.ActivationFunctionType
    OP = mybir.AluOpType

    nc = bacc.Bacc(dynamic_dma_scratch_size=65536, num_swdge_queues=4)
    blob_p = nc.declare_dram_parameter("blob", [1, nbytes], u8, isOutput=False)
    out_p = nc.declare_dram_parameter("out", [P, NB], f32, isOutput=True)

    xfull = nc.dram_tensor("xfull", [NROWS, 2 * FIN], u8, addr_space="Shared")
    t2own = nc.dram_tensor("t2own", [NOP, 512], u8)
    t2full = nc.dram_tensor("t2full", [NROWS, 512], u8, addr_space="Shared")
    d2sc = nc.dram_tensor("d2sc", [P, NB * H], bf16)

    blob_ap = blob_p[:]

    def bv(name, dt):
        off, parts, wbytes = layout[name]
        ap = bass.AP(tensor=blob_ap.tensor, offset=off,
                     ap=[[wbytes, parts], [1, wbytes]])
        return ap.bitcast(dt)

    def apx(base_ap, off, pattern):
        return bass.AP(tensor=base_ap.tensor, offset=base_ap.offset + off,
                       ap=[list(base_ap.ap[0])] + [list(q) for q in pattern])

    o_xs = layout["xs"][0]

    # ---------------- Block 0: AllGather x stripes -------------------------
    xs_ap = bass.AP(tensor=blob_ap.tensor, offset=o_xs,
                    ap=[[2 * FIN, NOP], [1, 2 * FIN]])
    xsown = nc.dram_tensor("xsown", [NOP, 2 * FIN], u8)
    with (nc.Block() as block, nc.semaphore("ccx") as ccx,
          nc.semaphore("xcp") as xcp):
        @block.gpsimd
        def _(gp):
            gp.dma_start(out=xsown[:], in_=xs_ap).then_inc(xcp, 16)
            gp.wait_ge(xcp, 16)
            gp.collective_compute(
                "AllGather", mybir.AluOpType.bypass,
                replica_groups=[list(range(NCORES))],
                ins=[xsown[:]], outs=[xfull[:]],
            ).then_inc(ccx)
            gp.wait_ge(ccx, 1)

    def load_consts(tc, ctxpool):
        idx_t = ctxpool.tile([P, Ttot * 8], i16, tag="idx")
        for k in range(8):
            nc.sync.dma_start(out=idx_t[16 * k:16 * (k + 1), :],
                              in_=bv("gidx", i16))
        dloc8 = ctxpool.tile([P, Ttot], u8, tag="dloc8")
        nc.sync.dma_start(out=dloc8[:], in_=bv("dloc", u8))
        dloc_t = ctxpool.tile([P, Ttot], bf16, tag="dloc")
        nc.vector.tensor_copy(dloc_t[:], dloc8[:])
        ioti = ctxpool.tile([P, P], mybir.dt.int32, tag="ioti")
        nc.gpsimd.iota(ioti[:], pattern=[[1, P]], base=0, channel_multiplier=0)
        iotab = ctxpool.tile([P, P], bf16, tag="iotab")
        nc.vector.tensor_copy(iotab[:], ioti[:])
        identb = ctxpool.tile([P, P], bf16, tag="identb")
        make_identity(nc, identb[:])
        return idx_t, dloc_t, iotab, identb

    def gathers(dest_ap_fn, idx_src, goff, ntiles, in_ap, elem, transpose,
                qn=0):
        done = 0
        while done < ntiles:
            nt = min(ntiles - done, GNT)
            K = nt * P
            col0 = (goff + done) * 8
            nc.gpsimd.dma_gather(
                out_ap=dest_ap_fn(done, nt),
                in_ap=in_ap,
                idxs_ap=idx_src[:, col0:col0 + nt * 8],
                num_idxs=K, num_idxs_reg=K, elem_size=elem,
                transpose=transpose, queue_num=qn)
            done += nt

    # ---------------- ctx 1: layer-1 edge pass -----------------------------
    with tile.TileContext(nc) as tc, ExitStack() as ctx:
        consts = ctx.enter_context(tc.tile_pool(name="c1", bufs=1))
        dpool = ctx.enter_context(tc.tile_pool(name="dp1", bufs=1))
        sb = ctx.enter_context(tc.tile_pool(name="s1", bufs=3))
        sb2 = ctx.enter_context(tc.tile_pool(name="s1b", bufs=2))
        pstp = ctx.enter_context(tc.tile_pool(name="ptp", bufs=3, space="PSUM"))
        psbig = ctx.enter_context(tc.tile_pool(name="pbig", bufs=3,
                                               space="PSUM"))
        pspo = ctx.enter_context(tc.tile_pool(name="ppo", bufs=2, space="PSUM"))

        idx_t, dloc_t, iotab, identb = load_consts(tc, consts)
        w1e = consts.tile([P, R1], bf16, tag="w1e")
        nc.sync.dma_start(out=w1e[:], in_=bv("w1e", bf16))
        w2e = consts.tile([P, 2, R2], bf16, tag="w2e")
        nc.sync.dma_start(out=w2e[:], in_=bv("w2e", bf16))
        sh1r = consts.tile([P, D1], f32, tag="sh1")
        nc.sync.dma_start(out=sh1r[:], in_=bv("sh1", f32))
        c2r = consts.tile([P, R2], f32, tag="c2")
        nc.sync.dma_start(out=c2r[:], in_=bv("c2", f32))
        d1a = consts.tile([P, NB * H], bf16, tag="d1a")
        nc.sync.dma_start(out=d1a[:], in_=bv("d1a", bf16))

        d2a = dpool.tile([P, NB * H], bf16, tag="d2a")

        xfull_bf = xfull[:].bitcast(bf16)
        xfullB_bf = xfull[SPLIT:NROWS, :].bitcast(bf16)

        for b in range(NB):
            TA = int(TAb[b]); TB = int(TBb[b])
            T = TA + TB
            o = int(toff[b])
            xTgA = sb.tile([P, TA * P], bf16, tag="xTgA")
            xTgB = sb.tile([P, TB * P], bf16, tag="xTgB")

            def xdstA(t0, nt, _x=xTgA):
                return apx(_x[:], t0 * P, [[1, 1], [1, nt * P]])

            def xdstB(t0, nt, _x=xTgB):
                return apx(_x[:], t0 * P, [[1, 1], [1, nt * P]])

            gathers(xdstA, idx_t, o, TA, xfull_bf, FIN, True, qn=b % 4)
            gathers(xdstB, idx_t, o + TA, TB, xfullB_bf, FIN, True,
                    qn=(b + 2) % 4)

            me = sb.tile([P, T * P], bf16, tag="me")
            nc.vector.tensor_tensor(
                out=apx(me[:], 0, [[P, T], [1, P]]),
                in0=apx(dloc_t[:], o, [[1, T], [0, P]]),
                in1=apx(iotab[:], 0, [[0, T], [1, P]]),
                op=OP.is_equal)

            G_all = sb.tile([P, T * R1], bf16, tag="G_all")
            po = pspo.tile([P, R1], f32, tag="po")
            meTa = sb.tile([P, T * P], bf16, tag="meTa")
            for t in range(T):
                tp = pstp.tile([P, P], bf16, tag="tp")
                nc.tensor.transpose(out=tp[:], in_=me[:, t * P:(t + 1) * P],
                                    identity=identb[:])
                nc.vector.tensor_copy(meTa[:, t * P:(t + 1) * P], tp[:])
            for t in range(T):
                Gps = psbig.tile([P, R1], f32, tag="Gps")
                xsrc = (xTgA[:, t * P:(t + 1) * P] if t < TA
                        else xTgB[:, (t - TA) * P:(t - TA + 1) * P])
                nc.tensor.matmul(out=Gps[:], lhsT=xsrc,
                                 rhs=w1e[:], start=True, stop=False)
                nc.tensor.matmul(out=Gps[:, D1:R1],
                                 lhsT=meTa[:, t * P:(t + 1) * P],
                                 rhs=d1a[:, b * H:(b + 1) * H],
                                 start=False, stop=True)
                nc.vector.tensor_copy(
                    G_all[:, t * R1:(t + 1) * R1], Gps[:])

            e1 = sb2.tile([P, T * H], bf16, tag="e1")
            nc.scalar.activation(e1[:], apx(G_all[:], D1, [[R1, T], [1, H]]),
                                 AF.Exp)
            e2 = sb2.tile([P, T * H], bf16, tag="e2")
            nc.scalar.activation(e2[:], apx(G_all[:], D1, [[R1, T], [1, H]]),
                                 AF.Exp, scale=0.2)
            ex = sb2.tile([P, T * H], bf16, tag="ex")
            nc.vector.tensor_tensor(out=ex[:], in0=e1[:], in1=e2[:], op=OP.max)
            we = sb2.tile([P, T * R1], bf16, tag="we")
            nc.vector.tensor_tensor(
                out=apx(we[:], 0, [[R1, T], [1, D1]]),
                in0=apx(G_all[:], 0, [[R1, T], [1, D1]]),
                in1=apx(ex[:], 0, [[H, T], [1, H], [0, C1]]), op=OP.mult)
            nc.vector.tensor_copy(
                out=apx(we[:], D1, [[R1, T], [1, H]]),
                in_=apx(ex[:], 0, [[H, T], [1, H]]))
            for t in range(T):
                nc.tensor.matmul(
                    out=po[:], lhsT=me[:, t * P:(t + 1) * P],
                    rhs=we[:, t * R1:(t + 1) * R1],
                    start=(t == 0), stop=(t == T - 1))

            den = sb2.tile([P, H], f32, tag="den")
            nc.vector.tensor_scalar_max(den[:], po[:, D1:R1], 1e-30)
            rden = sb2.tile([P, H], f32, tag="rden")
            nc.vector.reciprocal(rden[:], den[:])
            r = sb2.tile([P, D1], f32, tag="r")
            nc.vector.tensor_tensor(
                out=apx(r[:], 0, [[C1, H], [1, C1]]),
                in0=apx(po[:], 0, [[C1, H], [1, C1]]),
                in1=apx(rden[:], 0, [[1, H], [0, C1]]), op=OP.mult)
            nc.vector.tensor_add(r[:], r[:], sh1r[:])
            tneg = sb2.tile([P, D1], f32, tag="tneg")
            nc.vector.tensor_scalar_min(tneg[:], r[:], 0.0)
            texp = sb2.tile([P, D1], f32, tag="texp")
            nc.scalar.activation(texp[:], tneg[:], AF.Exp)
            ub = sb2.tile([P, D1], f32, tag="ub")
            nc.vector.tensor_scalar_max(ub[:], r[:], 0.0)
            ubb = sb2.tile([P, D1], bf16, tag="ubb")
            nc.vector.tensor_tensor(out=ubb[:], in0=ub[:], in1=texp[:],
                                    op=OP.add)
            uT = sb2.tile([P, 2, P], bf16, tag="uT")
            for k in range(2):
                tp = pstp.tile([P, P], bf16, tag="tp")
                nc.tensor.transpose(out=tp[:], in_=ubb[:, k * P:(k + 1) * P],
                                    identity=identb[:])
                nc.vector.tensor_copy(uT[:, k, :], tp[:])
            p2 = psbig.tile([P, R2], f32, tag="Gps")
            for k in range(2):
                nc.tensor.matmul(out=p2[:], lhsT=uT[:, k, :], rhs=w2e[:, k, :],
                                 start=(k == 0), stop=(k == 1))
            row2 = sb2.tile([P, R2], bf16, tag="row2")
            nc.vector.tensor_tensor(out=row2[:], in0=p2[:], in1=c2r[:],
                                    op=OP.add)
            nc.vector.tensor_copy(d2a[:, b * H:(b + 1) * H],
                                  row2[:, D2 + H:R2])
            t2o_ap = bass.AP(tensor=t2own[:].tensor, offset=b * P * 512,
                             ap=[[512, P], [1, 2 * R2]]).bitcast(bf16)
            nc.sync.dma_start(out=t2o_ap, in_=row2[:])

        nc.sync.dma_start(out=d2sc[:], in_=d2a[:])

    # ---------------- Block 1: AllGather t2 rows ---------------------------
    with (nc.Block() as block, nc.semaphore("cct") as cct):
        @block.gpsimd
        def _(gp):
            gp.collective_compute(
                "AllGather", mybir.AluOpType.bypass,
                replica_groups=[list(range(NCORES))],
                ins=[t2own[:]], outs=[t2full[:]],
            ).then_inc(cct)
            gp.wait_ge(cct, 1)

    # ---------------- ctx 2: layer-2 edge pass -----------------------------
    with tile.TileContext(nc) as tc, ExitStack() as ctx:
        consts = ctx.enter_context(tc.tile_pool(name="c2p", bufs=1))
        dpool = ctx.enter_context(tc.tile_pool(name="dp2", bufs=1))
        sb = ctx.enter_context(tc.tile_pool(name="s2", bufs=4))
        sb2 = ctx.enter_context(tc.tile_pool(name="s2b", bufs=3))
        pstp = ctx.enter_context(tc.tile_pool(name="ptp2", bufs=4, space="PSUM"))
        psz = ctx.enter_context(tc.tile_pool(name="psz", bufs=2, space="PSUM"))
        pspo = ctx.enter_context(tc.tile_pool(name="ppo2", bufs=2, space="PSUM"))

        idx_t, dloc_t, iotab, identb = load_consts(tc, consts)
        sh2r = consts.tile([P, D2], f32, tag="sh2")
        nc.sync.dma_start(out=sh2r[:], in_=bv("sh2", f32))
        fcwr = consts.tile([P, D2], f32, tag="fcw")
        nc.sync.dma_start(out=fcwr[:], in_=bv("fcw", f32))
        d2a = consts.tile([P, NB * H], bf16, tag="d2a")
        nc.sync.dma_start(out=d2a[:], in_=d2sc[:])
        out_all = dpool.tile([P, NB], f32, tag="out_all")

        t2_bf = t2full[:].bitcast(bf16)
        t2B_bf = t2full[SPLIT:NROWS, :].bitcast(bf16)

        for b in range(NB):
            TA = int(TAb[b]); TB = int(TBb[b])
            T = TA + TB
            o = int(toff[b])
            G2 = sb.tile([P, T, 256], bf16, tag="G2")

            def gdst(t0, nt, _g=G2):
                return _g[:, t0:t0 + nt, :]

            def gdstB(t0, nt, _g=G2, _ta=TA):
                return _g[:, _ta + t0:_ta + t0 + nt, :]

            gathers(gdst, idx_t, o, TA, t2_bf, 256, False, qn=b % 4)
            gathers(gdstB, idx_t, o + TA, TB, t2B_bf, 256, False, qn=b % 4)

            me = sb.tile([P, T * P], bf16, tag="me2")
            nc.vector.tensor_tensor(
                out=apx(me[:], 0, [[P, T], [1, P]]),
                in0=apx(dloc_t[:], o, [[1, T], [0, P]]),
                in1=apx(iotab[:], 0, [[0, T], [1, P]]),
                op=OP.is_equal)

            z2 = sb2.tile([P, T * H], bf16, tag="z2")
            po2 = pspo.tile([P, R2 - H], f32, tag="po2")
            meTa = sb.tile([P, T * P], bf16, tag="meTa2")
            for t in range(T):
                tp = pstp.tile([P, P], bf16, tag="tp")
                nc.tensor.transpose(out=tp[:], in_=me[:, t * P:(t + 1) * P],
                                    identity=identb[:])
                nc.vector.tensor_copy(meTa[:, t * P:(t + 1) * P], tp[:])
            zps = psz.tile([P, T * H], f32, tag="zps")
            for t in range(T):
                nc.tensor.matmul(out=zps[:, t * H:(t + 1) * H],
                                 lhsT=meTa[:, t * P:(t + 1) * P],
                                 rhs=d2a[:, b * H:(b + 1) * H],
                                 start=True, stop=True)
            nc.vector.tensor_tensor(
                out=z2[:], in0=zps[:],
                in1=apx(G2[:], D2, [[256, T], [1, H]]), op=OP.add)

            e1 = sb2.tile([P, T * H], bf16, tag="e1b")
            nc.scalar.activation(e1[:], z2[:], AF.Exp)
            e2 = sb2.tile([P, T * H], bf16, tag="e2b")
            nc.scalar.activation(e2[:], z2[:], AF.Exp, scale=0.2)
            ex = sb2.tile([P, T * H], bf16, tag="ex2")
            nc.vector.tensor_tensor(out=ex[:], in0=e1[:], in1=e2[:], op=OP.max)
            we = sb2.tile([P, T * (R2 - H)], bf16, tag="we2")
            nc.vector.tensor_tensor(
                out=apx(we[:], 0, [[R2 - H, T], [1, D2]]),
                in0=apx(G2[:], 0, [[256, T], [1, D2]]),
                in1=apx(ex[:], 0, [[H, T], [1, H], [0, C2]]), op=OP.mult)
            nc.vector.tensor_copy(
                out=apx(we[:], D2, [[R2 - H, T], [1, H]]),
                in_=apx(ex[:], 0, [[H, T], [1, H]]))
            for t in range(T):
                nc.tensor.matmul(
                    out=po2[:], lhsT=me[:, t * P:(t + 1) * P],
                    rhs=we[:, t * (R2 - H):(t + 1) * (R2 - H)],
                    start=(t == 0), stop=(t == T - 1))

            den = sb2.tile([P, H], f32, tag="den2")
            nc.vector.tensor_scalar_max(den[:], po2[:, D2:D2 + H], 1e-30)
            rden = sb2.tile([P, H], f32, tag="rden2")
            nc.vector.reciprocal(rden[:], den[:])
            r = sb2.tile([P, D2], f32, tag="r2")
            nc.vector.tensor_tensor(
                out=apx(r[:], 0, [[C2, H], [1, C2]]),
                in0=apx(po2[:], 0, [[C2, H], [1, C2]]),
                in1=apx(rden[:], 0, [[1, H], [0, C2]]), op=OP.mult)
            nc.vector.tensor_add(r[:], r[:], sh2r[:])
            tneg = sb2.tile([P, D2], f32, tag="tneg2")
            nc.vector.tensor_scalar_min(tneg[:], r[:], 0.0)
            texp = sb2.tile([P, D2], f32, tag="texp2")
            nc.scalar.activation(texp[:], tneg[:], AF.Exp)
            u2 = sb2.tile([P, D2], f32, tag="u2")
            nc.vector.tensor_scalar_max(u2[:], r[:], 0.0)
            nc.vector.tensor_add(u2[:], u2[:], texp[:])
            prodt = sb2.tile([P, D2], f32, tag="prodt")
            nc.vector.tensor_tensor(out=prodt[:], in0=u2[:], in1=fcwr[:],
                                    op=OP.mult)
            nc.vector.tensor_reduce(out_all[:, b:b + 1], prodt[:],
                                    axis=mybir.AxisListType.X, op=OP.add)

        outf = dpool.tile([P, NB], f32, tag="outf")
        nc.vector.tensor_scalar_add(outf[:], out_all[:], float(cfc))
        nc.sync.dma_start(out=out_p[:], in_=outf[:])

    nc.finalize()
    return nc


def run_spmd(nc, in_maps):
    from concourse.bass_utils import run_bass_kernel_spmd
    res = run_bass_kernel_spmd(nc, in_maps, core_ids=list(range(NCORES)))
    return res.results


def kernel(**inputs):
    in_maps, meta, layout, nbytes, cfc = preprocess(**inputs)
    nc = build_module(meta, layout, nbytes, cfc)
    results = run_spmd(nc, in_maps)
    out = np.empty((N, 1), np.float32)
    for c in range(NCORES):
        o = results[c]["out"]                 # [128, 49] partition-major
        y = np.ascontiguousarray(o.T).reshape(NOP)
        out[c * NO:(c + 1) * NO, 0] = y[:NO]
    return out
